# revision 39
# baseline (speedup 1.0000x reference)
"""Causal self-attention (B=2, S=2048, E=2048, H=16, D=128) with RoPE,
tensor-parallel over 8 TRN2 NeuronCores (2 heads per core).

Design (v2, software-pipelined):
- Whole SBUF datapath in fp16 (10-bit mantissa; ~5e-4 final rel err),
  PSUM accumulation in f32, outputs stored f32 straight from PSUM.
- Per-core work split into 8 segments s=(b, tt) of 512 tokens. Per
  segment three streams are interleaved instruction-by-instruction so
  the PE never waits: QKV projection of segment s, attention of
  segment s-1, out-projection of segment s-2.
- Causal masking: score/exp/PV/acc column ranges are trimmed per
  128-key chunk (boundary chunk r covers only queries >= 128r); the
  remaining 128x128 diagonal triangle is masked by one resident
  triangular f16 mask on the Pool engine.
- RoPE: rotate-half done by two partition-strided SBUF->SBUF DMAs per
  segment (sign folded into sinT on host); cos/sin multiplies split
  between Pool and DVE.
- Softmax: exp on Act engine (no max subtraction; scores ~ N(0,1)),
  denominator via ones-vector matmul of the f16 running sum (DVE),
  reciprocal on DVE, broadcast via rank-1 matmul, normalize on DVE.
- Each core emits a partial [E, B*S] f32 output through its Wout
  column-slice; the host sums the 8 partials and transposes back.
"""

import contextlib

import numpy as np

import concourse.bass as bass
import concourse.bacc as bacc
import concourse.tile as tile
import concourse.mybir as mybir
from concourse import bass_utils

B, S, E, H = 2, 2048, 2048, 16
D = E // H  # 128
NCORES = 8
HPC = H // NCORES  # heads per core = 2
T = B * S  # 4096 tokens
ROPE_BASE = 10000.0
P = 128
TT = 512  # token tile (free dim of most matmuls)
NTT = S // TT  # token tiles per batch = 4
NSEG = B * NTT  # 8 segments
NC_E = E // P  # contraction chunks over E = 16
SCALE = float(D) ** -0.5

f32 = mybir.dt.float32
f16 = mybir.dt.float16
EXP = mybir.ActivationFunctionType.Exp


def _merge_lists(lists):
    """Merge several unit lists into one, fractional-progress order."""
    out = []
    lists = [list(x) for x in lists if x]
    done = [0] * len(lists)
    total = sum(len(x) for x in lists)
    for _ in range(total):
        i = min((i for i in range(len(lists)) if done[i] < len(lists[i])),
                key=lambda i: done[i] / len(lists[i]))
        out.append(lists[i][done[i]])
        done[i] += 1
    return out


def _merge_streams(streams):
    """Emit units from several streams, keeping fractional progress even.

    A unit is a callable, or a (gate, callable) pair: the unit is not
    emitted until gate() is true (used to keep readers emitted after
    their writers -- the only ordering the tile framework tracks).
    """
    norm = [list(st) for st in streams if st]
    done = [0] * len(norm)
    grand = sum(len(u) for u in norm)
    emitted = 0
    while emitted < grand:
        best, bestv = -1, 2.0
        for i, units in enumerate(norm):
            if done[i] < len(units):
                nxt = units[done[i]]
                if isinstance(nxt, tuple) and not nxt[0]():
                    continue
                v = done[i] / len(units)
                if v < bestv:
                    best, bestv = i, v
        if best < 0:
            raise RuntimeError("merge stuck: all streams gated")
        nxt = norm[best][done[best]]
        (nxt[1] if isinstance(nxt, tuple) else nxt)()
        done[best] += 1
        emitted += 1


def _build_kernel(nc, tc, aps):
    xT, wqk, wv, wout, cosT, sinT, tri, rotm, o128, o1, outT = aps

    ctx = contextlib.ExitStack()
    with ctx:
        ctx.enter_context(nc.allow_low_precision(
            reason="fp16 datapath is intentional (tolerance 2e-2)"))
        const = ctx.enter_context(tc.tile_pool(name="const", bufs=1))
        sb = ctx.enter_context(tc.tile_pool(name="sb", bufs=2))
        ps = ctx.enter_context(tc.tile_pool(name="ps", bufs=1, space="PSUM"))

        # ---- resident constants --------------------------------------
        wqk_sb = const.tile([P, NC_E, 4 * P], f16)   # q0|q1|k0|k1 cols
        wv_sb = const.tile([P, NC_E, HPC * D], f16)
        wout_sb = const.tile([P, HPC, E], f16)
        cos_sb = const.tile([P, S], f16)
        sin_sb = const.tile([P, S], f16)
        tri_sb = const.tile([P, P], f16)
        rot_sb = const.tile([P, P], f16)
        o128_sb = const.tile([P, 1], f16)
        o1_sb = const.tile([1, P], f16)

        # ---- persistent tiles ----------------------------------------
        rope_done = {}  # s -> count of rope units emitted
        kr = {}     # (b, hl) -> [P, S] f16
        vt = {}     # b -> [P, 16, HPC*D] f16
        qr = {}     # s -> [P, HPC, TT] f16
        ctxt = {}   # (b, hl, j) -> [P, TT] f16
        fin_done = {}  # s -> count of fin units emitted
        xs = {}     # s -> [P, NC_E, TT] f16 (s=0: list of chunk tiles)
        for b in range(B):
            vt[b] = sb.tile([P, NC_E, HPC * D], f16, tag="vt", bufs=2,
                            name=f"vt_{b}")
            for hl in range(HPC):
                kr[(b, hl)] = sb.tile([P, S], f16, tag="kr", bufs=4,
                                      name=f"kr_{b}_{hl}")

        xTr = xT.rearrange("(c p) t -> p c t", p=P)

        def load_x(s, part, nparts=4):
            """DMA chunks [part*4, part*4+4) of segment s's x tile."""
            b, tt = divmod(s, NTT)
            col0 = b * S + tt * TT
            if s not in xs:
                xs[s] = sb.tile([P, NC_E, TT], f16, tag="x", bufs=2,
                                name=f"x_{s}")
            c0 = part * nparts
            nc.sync.dma_start(
                xs[s][:, c0:c0 + nparts, :],
                xTr[:, c0:c0 + nparts, col0:col0 + TT])

        # ============== stream builders ===============================

        def qkv_units(s):
            """fb-outer QKV + rope for segment s (x resident)."""
            b, tt = divmod(s, NTT)
            xt = xs[s]
            raw = [sb.tile([P, TT], f16, tag="raw", bufs=8,
                           name=f"raw_{s}_{i}") for i in range(4)]
            qr[s] = sb.tile([P, HPC, TT], f16, tag="qr", bufs=2,
                            name=f"qr_{s}")
            units = []
            pq_t = [None]
            qrot = [None] * 4

            def mk_qk(fb, quarter):
                def u():
                    if quarter == 0:
                        pq_t[0] = ps.tile([P, TT], f32, tag="qk", bufs=1,
                                          name=f"pq_{s}_{fb}")
                        if s + 1 < NSEG:
                            load_x(s + 1, fb)
                    pq = pq_t[0]
                    for c in range(quarter * 4, quarter * 4 + 4):
                        nc.tensor.matmul(
                            pq[:], wqk_sb[:, c, fb * P:(fb + 1) * P],
                            xt[:, c, :], start=(c == 0), stop=(c == NC_E - 1))
                    if quarter == 3:
                        nc.scalar.copy(raw[fb][:], pq[:])
                        # rotate-half via signed permutation, overwriting
                        # the same psum bank (ordered after the copy)
                        nc.tensor.matmul(pq[:], rot_sb[:], raw[fb][:],
                                         start=True, stop=True)
                        qrot[fb] = pq
                return u




            def mk_v(sub, half):
                def u():
                    if half == 0:
                        pq_t[0] = ps.tile([P, HPC * D], f32, tag="v",
                                          bufs=1, name=f"pv_{s}_{sub}")
                    pv = pq_t[0]
                    for c in range(half * 8, half * 8 + 8):
                        nc.tensor.matmul(
                            pv[:], xt[:, c, sub * P:(sub + 1) * P],
                            wv_sb[:, c, :], start=(c == 0),
                            stop=(c == NC_E - 1))
                    if half == 1:
                        nc.vector.tensor_copy(vt[b][:, tt * 4 + sub, :],
                                              pv[:])
                return u

            def mk_rope(fb):
                def u():
                    rope_done[s] = rope_done.get(s, 0) + 1
                    cs = slice(tt * TT, (tt + 1) * TT)
                    is_q, hl = fb < HPC, fb % HPC
                    t1 = sb.tile([P, TT], f16, tag="t1", bufs=2,
                                 name=f"t1_{s}_{fb}")
                    nc.vector.tensor_mul(t1[:], raw[fb][:],
                                          cos_sb[:, cs])
                    dst = qr[s][:, hl, :] if is_q else kr[(b, hl)][:, cs]
                    nc.vector.tensor_mul(dst, qrot[fb][:], sin_sb[:, cs])
                    nc.vector.tensor_add(dst, dst, t1[:])
                return u

            for fb in range(4):
                for quarter in range(4):
                    units.append(mk_qk(fb, quarter))
            vu = [mk_v(sub, half) for sub in range(4) for half in range(2)]
            units.append(vu[0])
            for fb in range(4):
                units.append(mk_rope(fb))
            units.extend(vu[1:])
            return units

        def attn_units(s):
            """Attention for segment s, as (early, late) streams: early =
            non-diagonal chunks (need kr/vt only through segment s-1 plus
            qr(s), so they can run in slot s), late = diagonal + finalize
            (runs in slot s+1)."""
            b, j = divmod(s, NTT)
            nch = 4 * j + 4
            early, late = [], []

            for hl in range(HPC):
                acc = sb.tile([P, TT], f16, tag="acc", bufs=4,
                              name=f"acc_{s}_{hl}")
                ct = sb.tile([P, TT], f16, tag="ctx", bufs=4,
                             name=f"ctx_{b}_{hl}_{j}")
                ctxt[(b, hl, j)] = ct
                pctx_t = [None]
                ex_t = {}

                def mk_slot(hl, psc_c, pv_c, acc=acc, pctx_t=pctx_t,
                            ex_t=ex_t):
                    def u():
                        if psc_c is not None:
                            c = psc_c
                            if c == 0:
                                pctx_t[0] = ps.tile([P, TT], f32,
                                                    tag="ctxp", bufs=2,
                                                    name=f"pctx_{s}_{hl}")
                            r = c - 4 * j  # boundary index (>=0: diagonal)
                            off = 128 * r if r > 0 else 0
                            w = slice(off, TT)
                            psc = ps.tile([P, TT], f32, tag="sc", bufs=2,
                                          name=f"psc_{s}_{hl}_{c}")
                            nc.tensor.matmul(
                                psc[:, w], kr[(b, hl)][:, c * P:(c + 1) * P],
                                qr[s][:, hl, w], start=True, stop=True)
                            ex = sb.tile([P, TT], f16, tag="ex", bufs=6,
                                         name=f"ex_{s}_{hl}_{c}")
                            ex_t[c] = ex
                            nc.scalar.activation(ex[:, w], psc[:, w], EXP,
                                                 scale=SCALE)
                            if r >= 0:
                                tw = slice(128 * r, 128 * (r + 1))
                                nc.gpsimd.tensor_mul(ex[:, tw], ex[:, tw],
                                                     tri_sb[:])
                        if pv_c is not None:
                            c = pv_c
                            r = c - 4 * j
                            off = 128 * r if r > 0 else 0
                            w = slice(off, TT)
                            ex = ex_t.pop(c)
                            nc.tensor.matmul(
                                pctx_t[0][:, w],
                                vt[b][:, c, hl * D:(hl + 1) * D], ex[:, w],
                                start=(c == 0), stop=(c == nch - 1))
                            if c == 0:
                                nc.vector.tensor_copy(acc[:], ex[:])
                            else:
                                nc.vector.tensor_add(acc[:, w], acc[:, w],
                                                     ex[:, w])
                    return u

                LAG = 3
                if j > 0:
                    cut = 4 * j

                    def gate(hl=hl):
                        return rope_done.get(s, 0) >= hl + 1
                    for c in range(cut + LAG):
                        early.append((gate, mk_slot(
                            hl, c if c < cut else None,
                            c - LAG if c >= LAG else None)))
                    for kk in range(cut, nch + LAG):
                        late.append(mk_slot(
                            hl, kk if kk < nch else None,
                            kk - LAG if kk - LAG >= cut else None))
                else:
                    for k in range(nch + LAG):
                        late.append(mk_slot(
                            hl, k if k < nch else None,
                            k - LAG if k >= LAG else None))

                def fin(hl=hl, acc=acc, ct=ct, pctx_t=pctx_t):
                    fin_done[s] = fin_done.get(s, 0) + 1
                    lp = ps.tile([P, TT], f32, tag="sc", bufs=2,
                                 name=f"lp_{s}_{hl}")
                    nc.tensor.matmul(lp[0:1, :], o128_sb[:], acc[:],
                                     start=True, stop=True)
                    lb = sb.tile([1, TT], f16, tag="lb", bufs=2,
                                 name=f"lb_{s}_{hl}")
                    nc.vector.reciprocal(lb[:], lp[0:1, :])
                    bp = ps.tile([P, TT], f32, tag="sc", bufs=2,
                                 name=f"bp_{s}_{hl}")
                    nc.tensor.matmul(bp[:], o1_sb[:], lb[:],
                                     start=True, stop=True)
                    lbb = sb.tile([P, TT], f16, tag="lbb", bufs=2,
                                  name=f"lbb_{s}_{hl}")
                    nc.scalar.copy(lbb[:], bp[:])
                    nc.vector.tensor_mul(ct[:], pctx_t[0][:], lbb[:])
                late.append(fin)
            return early, late

        def outproj_units(s, tags=None):
            b, j = divmod(s, NTT)
            col0 = b * S + j * TT
            units = []
            ost_t = [None]
            tags = tags or [("po", 2)]
            scr = sb.tile([1, 2], f16, tag="scr", bufs=2,
                          name=f"scr_{s}")

            def mk(of):
                def u():
                    tg, tb = tags[(of // 2) % len(tags)]
                    if of % 2 == 0:
                        ost_t[0] = ps.tile([P, TT], f32, tag=tg, bufs=tb,
                                           name=f"po_{s}_{of}"), \
                            sb.tile([P, 2, TT], f16, tag="ost", bufs=8,
                                    name=f"ost_{s}_{of}")
                    po, ost = ost_t[0]
                    if of % 2 == 1:
                        po = ps.tile([P, TT], f32, tag=tg, bufs=tb,
                                     name=f"po_{s}_{of}")
                    for hl in range(HPC):
                        nc.tensor.matmul(
                            po[:], wout_sb[:, hl, of * P:(of + 1) * P],
                            ctxt[(b, hl, j)][:],
                            start=(hl == 0), stop=(hl == HPC - 1))
                    dve_pair = (of // 2) % 2 == 1
                    if dve_pair:
                        nc.vector.tensor_copy(ost[:, of % 2, :], po[:])
                    else:
                        nc.scalar.copy(ost[:, of % 2, :], po[:])
                    if of % 2 == 1:
                        if dve_pair:
                            # Act touch of the DVE-written half: gives the
                            # scalar-queue store a tracked ordering
                            nc.scalar.copy(scr[0:1, :], ost[0:1, 1, 0:2])
                        nc.scalar.dma_start(
                            outT.rearrange("(c p) t -> p c t", p=P)
                                [:, of - 1:of + 1, col0:col0 + TT],
                            ost[:])
                return u
            for of in range(E // P):
                units.append(mk(of))
            return units

        # ============== segment 0: cold start =========================
        # Interleave per-chunk weight + x DMAs on two queues, consume
        # c-outer so the PE starts ~2us in and stays busy.
        # PE warm-up: ramp the clock while the first DMAs are in flight
        dum = sb.tile([P, TT], f16, tag="t1", bufs=2, name="dum")
        nc.vector.memset(dum[:], 0.5)
        pdum = ps.tile([P, TT], f32, tag="sc", bufs=2, name="pdum")
        for i in range(14):
            nc.tensor.matmul(pdum[:], dum[:, 0:P], dum[:],
                             start=True, stop=True)

        x0t = sb.tile([P, NC_E, TT], f16, tag="x", bufs=2, name="x_0")
        xs0 = [x0t[:, c, :] for c in range(NC_E)]
        wqkr = wqk.rearrange("(c p) f -> p c f", p=P)
        x0r = xTr[:, :, 0:TT]
        for a, bnd in [(0, 1), (1, 2), (2, 4), (4, 6), (6, 9), (9, 12),
                       (12, 16)]:
            gs = slice(a, bnd)
            nc.scalar.dma_start(wqk_sb[:, gs, :], wqkr[:, gs, :])
            nc.sync.dma_start(x0t[:, gs, :], x0r[:, gs, :])
        # rot matrix is read by segment 0's rotate matmuls: load FIRST
        nc.gpsimd.dma_start(rot_sb[:], rotm)
        # bulk/later constants behind the startup wave
        wvr = wv.rearrange("(c p) f -> p c f", p=P)
        nc.sync.dma_start(wv_sb[:, 0:8, :], wvr[:, 0:8, :])
        nc.scalar.dma_start(wv_sb[:, 8:16, :], wvr[:, 8:16, :])
        nc.sync.dma_start(cos_sb[:], cosT)
        nc.scalar.dma_start(sin_sb[:], sinT)

        # c-outer qk accumulation for segment 0
        raw0 = [sb.tile([P, TT], f16, tag="raw", bufs=8,
                        name=f"raw_0_{i}") for i in range(4)]
        qr[0] = sb.tile([P, HPC, TT], f16, tag="qr", bufs=2, name="qr_0")
        qk0 = [ps.tile([P, TT], f32, tag=tg, bufs=bf, name=f"pq0_{fb}")
               for fb, (tg, bf) in enumerate(
                   [("qk", 1), ("sc", 2), ("sc", 2), ("ctxp", 2)])]
        for c in range(NC_E):
            for fb in range(4):
                nc.tensor.matmul(
                    qk0[fb][:], wqk_sb[:, c, fb * P:(fb + 1) * P],
                    xs0[c], start=(c == 0), stop=(c == NC_E - 1))
        for fb in range(4):
            nc.scalar.copy(raw0[fb][:], qk0[fb][:])
        for fb in range(4):
            nc.tensor.matmul(qk0[fb][:], rot_sb[:], raw0[fb][:],
                             start=True, stop=True)
        # later-phase constants via SWDGE, behind the startup wave
        for hl in range(HPC):
            nc.gpsimd.dma_start(wout_sb[:, hl, :],
                                wout[hl * P:(hl + 1) * P, :])
        nc.gpsimd.dma_start(tri_sb[:], tri)
        nc.gpsimd.dma_start(o128_sb[:], o128)
        nc.gpsimd.dma_start(o1_sb[:], o1)
        # v chains for segment 0: alternate psum tags so chains overlap
        for sub in range(4):
            if sub % 2 == 0:
                pv0 = ps.tile([P, HPC * D], f32, tag="v", bufs=1,
                              name=f"pv0_{sub}")
                pva = pv0[:]
            else:
                pv0 = ps.tile([P, TT], f32, tag="ctxp", bufs=2,
                              name=f"pv0_{sub}")
                pva = pv0[:, 0:HPC * D]
            for c in range(NC_E):
                nc.tensor.matmul(pva, xs0[c][:, sub * P:(sub + 1) * P],
                                 wv_sb[:, c, :], start=(c == 0),
                                 stop=(c == NC_E - 1))
            nc.vector.tensor_copy(vt[0][:, sub, :], pva)
        # rope for segment 0
        for fb in range(4):
            is_q, hl = fb < HPC, fb % HPC
            t1 = sb.tile([P, TT], f16, tag="t1", bufs=2, name=f"t1_0_{fb}")
            nc.vector.tensor_mul(t1[:], raw0[fb][:], cos_sb[:, 0:TT])
            dst = qr[0][:, hl, :] if is_q else kr[(0, hl)][:, 0:TT]
            nc.vector.tensor_mul(dst, qk0[fb][:], sin_sb[:, 0:TT])
            nc.vector.tensor_add(dst, dst, t1[:])
        load_x(1, 0)
        load_x(1, 1)
        load_x(1, 2)
        load_x(1, 3)

        # ============== segments 1..7 + drain =========================
        pend_late = attn_units(0)[1]  # attn(0) is all-diagonal -> slot 1
        for s in range(1, NSEG + 1):
            streams = []
            att = []
            if pend_late is not None:
                att += pend_late
                pend_late = None
            if s < NSEG:
                streams.append(qkv_units(s))
                early, pend_late = attn_units(s)
                att += early
            if att:
                streams.append(att)
            if s - 2 >= 0:
                streams.append(outproj_units(s - 2))
            if s == NSEG:
                # drain: attn(7) late + outproj(6) + outproj(7) gated on
                # attn(7)'s finalize emission
                def oj7_gate():
                    return fin_done.get(NSEG - 1, 0) >= 2
                streams.append([(oj7_gate, u)
                                for u in outproj_units(
                                    NSEG - 1,
                                    tags=[("ctxp", 2), ("sc", 2)])])
            _merge_streams(streams)


def build_nc():
    nc = bacc.Bacc("TRN2", target_bir_lowering=False, debug=False,
                   num_devices=NCORES)
    xT = nc.dram_tensor("xT", [E, T], f16, kind="ExternalInput").ap()
    wqk = nc.dram_tensor("wqkT", [E, 4 * P], f16, kind="ExternalInput").ap()
    wv = nc.dram_tensor("wvT", [E, HPC * D], f16, kind="ExternalInput").ap()
    wout = nc.dram_tensor("woutT", [HPC * D, E], f16,
                          kind="ExternalInput").ap()
    cosT = nc.dram_tensor("cosT", [D, S], f16, kind="ExternalInput").ap()
    sinT = nc.dram_tensor("sinT", [D, S], f16, kind="ExternalInput").ap()
    tri = nc.dram_tensor("tri", [P, P], f16, kind="ExternalInput").ap()
    rotm = nc.dram_tensor("rotm", [P, P], f16, kind="ExternalInput").ap()
    o128 = nc.dram_tensor("o128", [P, 1], f16, kind="ExternalInput").ap()
    o1 = nc.dram_tensor("o1", [1, P], f16, kind="ExternalInput").ap()
    outT = nc.dram_tensor("outT", [E, T], f16, kind="ExternalOutput").ap()
    with tile.TileContext(nc) as tc:
        _build_kernel(nc, tc, (xT, wqk, wv, wout, cosT, sinT, tri,
                               rotm, o128, o1, outT))
    nc.compile()
    return nc


def host_inputs(x, Wqkv, Wout):
    """Per-core input dicts (numpy)."""
    fp16 = np.float16

    xT = np.ascontiguousarray(x.reshape(T, E).T).astype(fp16)

    inv_freq = 1.0 / (ROPE_BASE ** (np.arange(0, D, 2, dtype=np.float64) / D))
    pos = np.arange(S, dtype=np.float64)
    freqs = np.outer(pos, inv_freq)            # [S, D/2]
    ang = np.concatenate([freqs, freqs], -1)   # [S, D]
    cosT = np.ascontiguousarray(np.cos(ang).T).astype(fp16)
    sinT = np.ascontiguousarray(np.sin(ang).T).astype(fp16)
    rotm = np.zeros((P, P), fp16)
    ii = np.arange(0, D, 2)
    rotm[ii + 1, ii] = -1.0   # out[2i]   = -in[2i+1]
    rotm[ii, ii + 1] = 1.0    # out[2i+1] =  in[2i]

    tri = (np.arange(P)[:, None] <= np.arange(P)[None, :]).astype(fp16)
    o128 = np.ones((P, 1), fp16)
    o1 = np.ones((1, P), fp16)

    in_maps = []
    for core in range(NCORES):
        r0 = HPC * D * core  # 256*core
        wq = Wqkv[r0:r0 + HPC * D]               # [256, E] rows q_h0|q_h1
        wk = Wqkv[E + r0:E + r0 + HPC * D]
        wv_ = Wqkv[2 * E + r0:2 * E + r0 + HPC * D]
        wqkT = np.ascontiguousarray(
            np.concatenate([wq, wk], 0).T).astype(fp16)   # [E, 512]
        wvT = np.ascontiguousarray(wv_.T).astype(fp16)    # [E, 256]
        woutT = np.ascontiguousarray(
            Wout[:, r0:r0 + HPC * D].T).astype(fp16)      # [256, E]
        in_maps.append({
            "xT": xT, "wqkT": wqkT, "wvT": wvT, "woutT": woutT,
            "cosT": cosT, "sinT": sinT, "tri": tri, "rotm": rotm,
            "o128": o128, "o1": o1,
        })
    return in_maps


_NC_CACHE = None


def kernel(x, Wqkv, Wout):
    global _NC_CACHE
    x = np.asarray(x)
    Wqkv = np.asarray(Wqkv)
    Wout = np.asarray(Wout)
    in_maps = host_inputs(x, Wqkv, Wout)
    if _NC_CACHE is None:
        _NC_CACHE = build_nc()
    res = bass_utils.run_bass_kernel_spmd(
        _NC_CACHE, in_maps, core_ids=list(range(NCORES)))
    acc = np.zeros((E, T), np.float64)
    for c in range(NCORES):
        acc += res.results[c]["outT"].astype(np.float64)
    out = acc.T.reshape(B, S, E).astype(np.float32)
    return out
